# revision 3
# baseline (speedup 1.0000x reference)
"""Pairwise-interaction kernel for Trainium2 (raw Bass), 8-core SPMD.

Computes out[b, p, :] = x[b, i(p), :] * x[b, j(p), :] for all pairs
(i < j) of the F=26 feature rows, p ordered row-major (i outer, j inner).

Sharding: data-parallel over the batch dim (16384 -> 8 x 2048), no
cross-device communication.

Design (f32 v1 was ~221us; bf16 single-ring v2 ~119us; this v3 targets
the descriptor-flow stalls seen in the v2 trace):
  * All tensors bf16: DVE 2x packing doubles tensor_tensor throughput
    vs f32 AND halves HBM traffic to 42.6MB stores + 3.4MB loads per
    core. Added rounding error (~1.1e-2 measured) is inside the 2e-2
    gate; f32<->bf16 conversion happens on the host.
  * Samples interleaved G=4 per partition row (sample = t*P*G + p*G+g):
    every TT instruction covers all 4 groups via a [P, G, nrep, D]
    broadcast AP (DVE ~95us busy), and each DMA descriptor row is a
    multi-KB contiguous DRAM run.
  * v2's trace showed the kernel is bound by SDMA descriptor FLOW, not
    bandwidth: all 25 store triggers rode the single SP HWDGE ring, so
    a ring-full DIRECT2D stall on one chunk (10-15us observed) blocked
    generation of every later chunk. The 16 SDMA engines drained the
    ring and sat IDLE 102-110us while the last supertile's descriptors
    were stuck behind the stall, then chewed a 15us post-compute tail
    (end 125.8us). Fix: alternate store triggers between the TWO
    physical HWDGE rings (SP ring = nc.sync, Act ring = nc.scalar; the
    engines round-robin rings at packet granularity), so one ring's
    reclamation stall never starves the engines - the other ring keeps
    feeding them. Completion tracking splits into one semaphore per
    ring (per-ring FIFO completion order still holds).
  * Loads ride the Act ring ahead of the odd store triggers (all 5
    load DIRECT2Ds are generated by ~10.7us, before the first store
    trigger releases), so a load never queues behind a multi-MB store.
  * Store chunks are pair-ranges sized tiny/huge/small (25/164/100/26/
    10 pairs) so the store stream starts ~2us into the first sweep and
    the final sweep ends on the smallest chunks.
  * All NTS=4 input loads are issued up-front (XB=NTS buffers, 27KB of
    SBUF); supertile 0 is split into two half-group (GS=2) loads and
    sweeps so the first store chunk is ready earlier.

Raw-Bass sync scheme (one semaphore wait per instruction; extra
ordering uses standalone wait_ge ops on the engine queue):
  sem_ld (+16 per load DMA, Act ring)
  sem_sa (+16 per even store, SP ring) / sem_sb (+16 per odd store,
    Act ring); store s of pass p chunk c has s = NCH*p + c, rides
    queue s%2, and is that queue's (s//2+1)-th DMA.
  sem_tt (+1 by the last TT of each chunk, vector engine)
"""

import numpy as np
import ml_dtypes

import concourse.bass as bass
from concourse import mybir
from concourse.bass_utils import run_bass_kernel_spmd

B, F, D = 16384, 26, 32
NCORES = 8
BC = B // NCORES           # 2048 samples per core
P = 128                    # SBUF partitions
G = 4                      # sample groups per supertile (consecutive rows)
GS = G // 2                # groups per half-sweep (supertile 0 only)
NTS = BC // (P * G)        # 4 supertiles per core
FD = F * D                 # 832
NPAIR = F * (F - 1) // 2   # 325
OD = NPAIR * D             # 10400

XB = NTS                   # all input supertiles resident at once
YB = 2                     # output supertile buffers

# pair-index chunk boundaries (25, 164, 100, 26, 10 pairs): tiny first
# chunk so the store stream starts early; big middle chunks keep
# descriptor rows at 6.4-10.5KB (per-engine line rate); the trailing
# chunks shrink the post-compute drain.
CHUNKS = [(0, 1), (1, 9), (9, 17), (17, 21), (21, 25)]
NCH = len(CHUNKS)

# passes: (supertile t, g_lo, g_hi). Supertile 0 runs as two
# half-group passes so its first store chunk is ready early.
PASSES = [(0, 0, GS), (0, GS, G), (1, 0, G), (2, 0, G), (3, 0, G)]
NPASS = len(PASSES)

BF16 = mybir.dt.bfloat16
NP_BF16 = ml_dtypes.bfloat16


def _pair_off(i_lo):
    return sum(F - 1 - i for i in range(i_lo))


def _s_idx(p, c):
    # global store index: pass-major, chunk-minor (== sem_tt threshold-1)
    return NCH * p + c


def _s_queue(s):
    # 0 -> SP ring (nc.sync), 1 -> Act ring (nc.scalar)
    return s % 2


def _s_ord(s):
    # 1-based ordinal of store s within its ring
    return s // 2 + 1


_nc_cache = None


def _build_nc():
    nc = bass.Bass()
    x = nc.declare_dram_parameter("x", [BC, FD], BF16, isOutput=False)
    y = nc.declare_dram_parameter("y", [BC, OD], BF16, isOutput=True)
    xv = x[:].rearrange("(t p g) m -> t p (g m)", p=P, g=G)
    yv = y[:].rearrange("(t p g) m -> t p g m", p=P, g=G)

    with (
        nc.sbuf_tensor([P, XB * G * FD], BF16) as xbuf,
        nc.sbuf_tensor([P, YB * G * OD], BF16) as ybuf,
        nc.semaphore("sem_ld") as sem_ld,
        nc.semaphore("sem_sa") as sem_sa,
        nc.semaphore("sem_sb") as sem_sb,
        nc.semaphore("sem_tt") as sem_tt,
        nc.Block() as blk,
    ):
        xts = [xbuf[:, b * G * FD : (b + 1) * G * FD] for b in range(XB)]
        yts = [ybuf[:, b * G * OD : (b + 1) * G * OD] for b in range(YB)]
        st_sems = [sem_sa, sem_sb]

        def _pass_bufs(p):
            t, g_lo, g_hi = PASSES[p]
            yt = yts[t % YB].rearrange("p (g m) -> p g m", g=G)
            return t, g_lo, g_hi, yt

        def emit_stores(q, parity):
            # store triggers of one HWDGE ring, in global-s order
            for p in range(NPASS):
                t, g_lo, g_hi, yt = _pass_bufs(p)
                for c, (i_lo, i_hi) in enumerate(CHUNKS):
                    s = _s_idx(p, c)
                    if s % 2 != parity:
                        continue
                    p_lo, p_hi = _pair_off(i_lo), _pair_off(i_hi)
                    st = q.dma_start(
                        yv[t][:, g_lo:g_hi, p_lo * D : p_hi * D],
                        yt[:, g_lo:g_hi, p_lo * D : p_hi * D],
                    )
                    st._wait_ge(sem_tt, s + 1)
                    st.then_inc(st_sems[parity], 16)

        @blk.scalar
        def _(scalar):
            for h in range(2):
                scalar.dma_start(
                    xts[0][:, h * GS * FD : (h + 1) * GS * FD],
                    xv[0][:, h * GS * FD : (h + 1) * GS * FD],
                ).then_inc(sem_ld, 16)
            for t in range(1, NTS):
                scalar.dma_start(xts[t], xv[t]).then_inc(sem_ld, 16)
            emit_stores(scalar, 1)

        @blk.sync
        def _(sync):
            emit_stores(sync, 0)

        @blk.vector
        def _(v):
            for p in range(NPASS):
                t, g_lo, g_hi, yt = _pass_bufs(p)
                xt = xts[t].rearrange("p (g m) -> p g m", g=G)
                ng = g_hi - g_lo
                # loads land in pass order: h0, h1, t1, t2, t3
                v.wait_ge(sem_ld, 16 * (p + 1))
                for c, (i_lo, i_hi) in enumerate(CHUNKS):
                    # overwrite guards: ybuf[t%YB] chunk c must be
                    # stored out for every earlier pass that used it
                    if p >= 3:
                        prev = [_s_idx(0, c), _s_idx(1, c)] if p == 3 else [
                            _s_idx(2, c)
                        ]
                        for s in prev:
                            v.wait_ge(st_sems[_s_queue(s)], 16 * _s_ord(s))
                    off = _pair_off(i_lo)
                    for i in range(i_lo, i_hi):
                        nrep = F - 1 - i
                        in0 = (
                            xt[:, g_lo:g_hi, i * D : (i + 1) * D]
                            .unsqueeze(2)
                            .broadcast_to([P, ng, nrep, D])
                        )
                        in1 = xt[:, g_lo:g_hi, (i + 1) * D : FD].rearrange(
                            "p g (r d) -> p g r d", d=D
                        )
                        outap = yt[
                            :, g_lo:g_hi, off * D : (off + nrep) * D
                        ].rearrange("p g (r d) -> p g r d", d=D)
                        tt = nc.vector.tensor_mul(outap, in0, in1)
                        off += nrep
                    tt.then_inc(sem_tt, 1)

    return nc


def _make_in_maps(inputs: np.ndarray):
    x = np.asarray(inputs, dtype=np.float32).reshape(B, FD).astype(NP_BF16)
    shards = np.ascontiguousarray(x.reshape(NCORES, BC, FD))
    return [{"x": shards[c]} for c in range(NCORES)]


def kernel(inputs: np.ndarray) -> np.ndarray:
    global _nc_cache
    if _nc_cache is None:
        _nc_cache = _build_nc()
    nc = _nc_cache

    in_maps = _make_in_maps(inputs)
    res = run_bass_kernel_spmd(nc, in_maps, list(range(NCORES)))
    out = np.concatenate([res.results[c]["y"] for c in range(NCORES)], axis=0)
    return out.astype(np.float32).reshape(B, NPAIR, D)
